# revision 13
# baseline (speedup 1.0000x reference)
"""Trainium2 Bass kernel for a GQA sliding-window attention layer.

Reference computation (B=2, T=2048, C=2048, 16 Q heads / 4 KV heads, d=128):
    q = x @ Wq; k = x @ Wk; v = x @ Wv (+ sigmoid-gated value embedding)
    q, k = rmsnorm(rope(q)), rmsnorm(rope(k))
    scores masked to the band 0 <= j - i < window (=1024), softmax over j
    out = (p @ v) @ Wo

Sharding: 8 cores = 2 batches x 4 KV groups.  Each core computes its 4 Q
heads / 1 KV head for one batch and a partial output (its 512-row slice of
the Wo contraction); the host sums the 4 partials per batch.

Layout strategy per core:
  - xT (C x T, bf16) resident in SBUF; all projections contract over C.
  - q̂T / k̂T kept [d=128 partitions, T free]; scores computed transposed
    (S^T tiles [kj, qi]) so that P^T feeds the PV matmul directly with v in
    natural [token, d] layout (no P transposes).
  - softmax has no max-subtraction: rms-normalized q,k bound |score| by
    sqrt(128), so exp is safe in fp32.
  - per-q softmax denominators and rms rows are broadcast across partitions
    via a tiny DRAM bounce (SBUF APs need nonzero partition stride).
"""

import numpy as np
import ml_dtypes

BF16 = ml_dtypes.bfloat16

# Problem dims (hardcoded per contest rules)
B, T, C = 2, 2048, 2048
N_HEAD, N_KV, HD, GATE_CH = 16, 4, 128, 32
WINDOW = 1024
P = 128
GH = N_HEAD // N_KV  # q heads per kv head (= per core)
N_CORES = 8

_PROGRAM_CACHE = {}


def build_program(T_=T, C_=C, win=WINDOW):
    import concourse.mybir as mybir
    import concourse.tile as tile
    from concourse import bacc

    dt = mybir.dt
    f32 = dt.float32
    bf16 = dt.bfloat16
    AF = mybir.ActivationFunctionType
    ALU = mybir.AluOpType

    NT = T_ // P          # token tiles
    KT = C_ // P          # contraction tiles
    WT = win // P         # window tiles
    ISQ = 1.0 / float(np.sqrt(HD))

    nc = bacc.Bacc()

    xT = nc.declare_dram_parameter("xT", [C_, T_], bf16, isOutput=False)
    wq = nc.declare_dram_parameter("wq", [C_, GH * HD], bf16, isOutput=False)
    wk = nc.declare_dram_parameter("wk", [C_, HD], bf16, isOutput=False)
    wv = nc.declare_dram_parameter("wv", [C_, HD], bf16, isOutput=False)
    wg = nc.declare_dram_parameter("wg", [GATE_CH, 1], bf16, isOutput=False)
    ve2 = nc.declare_dram_parameter("ve2", [T_, HD], bf16, isOutput=False)
    wo = nc.declare_dram_parameter("wo", [GH * HD, C_], bf16, isOutput=False)
    ccd = nc.declare_dram_parameter("cc", [P, T_], bf16, isOutput=False)
    ssd = nc.declare_dram_parameter("ss", [P, T_], bf16, isOutput=False)
    tlo = nc.declare_dram_parameter("tlo", [P, P], bf16, isOutput=False)
    thi = nc.declare_dram_parameter("thi", [P, P], bf16, isOutput=False)
    out_d = nc.declare_dram_parameter("out", [T_, C_], f32, isOutput=True)

    with tile.TileContext(nc) as tc:
        with (
            tc.tile_pool(name="singles", bufs=1) as sg,
            tc.tile_pool(name="work", bufs=2) as wk_pool,
            tc.tile_pool(name="attw", bufs=4) as aw,
            tc.tile_pool(name="outp", bufs=3) as op_pool,
            tc.tile_pool(name="psum", bufs=8, space="PSUM") as pp,
            tc.tile_pool(name="drb", bufs=4, space="DRAM") as drp,
        ):
            # ---- persistent inputs -------------------------------------
            xt = []
            for kt in range(KT):
                t_ = sg.tile([P, T_], bf16, tag=f"xt{kt}")
                nc.sync.dma_start(out=t_[:], in_=xT[kt * P:(kt + 1) * P, :])
                xt.append(t_)
            wq_sb = sg.tile([P, KT, GH * HD], bf16, tag="wq")
            nc.sync.dma_start(out=wq_sb[:], in_=wq.rearrange("(o p) n -> p o n", p=P))
            wk_sb = sg.tile([P, KT, HD], bf16, tag="wk")
            nc.sync.dma_start(out=wk_sb[:], in_=wk.rearrange("(o p) n -> p o n", p=P))
            wv_sb = sg.tile([P, KT, HD], bf16, tag="wv")
            nc.sync.dma_start(out=wv_sb[:], in_=wv.rearrange("(o p) n -> p o n", p=P))
            wo_sb = sg.tile([P, GH, C_], bf16, tag="wo")
            nc.sync.dma_start(out=wo_sb[:], in_=wo.rearrange("(o p) n -> p o n", p=P))
            wg_sb = sg.tile([GATE_CH, 1], bf16, tag="wg")
            nc.sync.dma_start(out=wg_sb[:], in_=wg[:])
            cc_sb = sg.tile([P, T_], bf16, tag="cc")
            nc.sync.dma_start(out=cc_sb[:], in_=ccd[:])
            ss_sb = sg.tile([P, T_], bf16, tag="ss")
            nc.sync.dma_start(out=ss_sb[:], in_=ssd[:])
            ve2_sb = sg.tile([P, NT, HD], bf16, tag="ve2")
            nc.sync.dma_start(out=ve2_sb[:], in_=ve2.rearrange("(o p) d -> p o d", p=P))
            tlo_sb = sg.tile([P, P], bf16, tag="tlo")
            nc.sync.dma_start(out=tlo_sb[:], in_=tlo[:])
            thi_sb = sg.tile([P, P], bf16, tag="thi")
            nc.sync.dma_start(out=thi_sb[:], in_=thi[:])
            ones_sb = sg.tile([P, 1], bf16, tag="onesb")
            nc.vector.memset(ones_sb[:], 1.0)
            eps_sb = sg.tile([P, 1], f32, tag="epsb")
            nc.vector.memset(eps_sb[:], 1e-6)

            # persistent intermediates
            qhat = sg.tile([P, GH, T_], bf16, tag="qhat")   # normalized roped q, [d, h, t]
            khat = sg.tile([P, T_], bf16, tag="khat")       # normalized roped k * isq
            vsb = sg.tile([P, NT, HD], bf16, tag="vsb")     # gated v, [tok, tt, d]

            TS = T_ // 512  # 512-wide token slices

            # ---- projections + rope + rmsnorm for k and q heads --------
            # head index: 0 => K, 1..GH => Q h-1
            for head in range(GH + 1):
                is_k = head == 0
                for ts_ in range(TS):
                    sl = slice(ts_ * 512, ts_ * 512 + 512)
                    ps = pp.tile([P, 512], f32, tag="pb")
                    for kt in range(KT):
                        if is_k:
                            w_ap = wk_sb[:, kt, :]
                        else:
                            h = head - 1
                            w_ap = wq_sb[:, kt, h * HD:(h + 1) * HD]
                        nc.tensor.matmul(
                            ps[:], lhsT=w_ap, rhs=xt[kt][:, sl],
                            start=(kt == 0), stop=(kt == KT - 1),
                        )
                    # rope: qr = ps*cc + swap(ps)*ss   (ss carries the sign)
                    qr = wk_pool.tile([P, 512], f32, tag="qr")
                    nc.vector.tensor_mul(qr[:], ps[:], cc_sb[:, sl])
                    qs = wk_pool.tile([P, 512], f32, tag="qs")
                    nc.vector.tensor_mul(qs[0:64, :], ps[64:128, :], ss_sb[0:64, sl])
                    nc.vector.tensor_mul(qs[64:128, :], ps[0:64, :], ss_sb[64:128, sl])
                    nc.vector.tensor_add(qr[:], qr[:], qs[:])
                    # rms row: 1/sqrt(mean(qr^2)+eps)
                    q2 = wk_pool.tile([P, 512], bf16, tag="q2")
                    nc.vector.tensor_mul(q2[:], qr[:], qr[:])
                    ssq = pp.tile([1, 512], f32, tag="pb")
                    nc.tensor.matmul(ssq[:], lhsT=ones_sb[:], rhs=q2[:],
                                     start=True, stop=True)
                    srow = wk_pool.tile([1, 512], f32, tag="srow")
                    nc.scalar.activation(srow[:], ssq[:], AF.Sqrt,
                                         bias=eps_sb[0:1, :], scale=1.0 / HD)
                    rr = wk_pool.tile([1, 512], f32, tag="rr")
                    nc.vector.reciprocal(rr[:], srow[:])
                    if is_k:
                        # fold the 1/sqrt(d) score scale into k̂
                        nc.vector.tensor_scalar_mul(rr[:], rr[:], ISQ)
                    # broadcast rr across partitions via DRAM bounce
                    db = drp.tile([1, 512], f32, tag="bounce")
                    nc.sync.dma_start(out=db[:], in_=rr[:])
                    rrb = wk_pool.tile([P, 512], f32, tag="bcast")
                    nc.sync.dma_start(out=rrb[:], in_=db[:].to_broadcast((P, 512)))
                    dest = khat[:, sl] if is_k else qhat[:, head - 1, sl]
                    nc.vector.tensor_mul(dest, qr[:], rrb[:])

            # ---- v projection + sigmoid-gated value embedding ----------
            for tt in range(NT):
                tsl = slice(tt * P, (tt + 1) * P)
                vps = pp.tile([P, HD], f32, tag="pb")
                for kt in range(KT):
                    nc.tensor.matmul(
                        vps[:], lhsT=xt[kt][:, tsl], rhs=wv_sb[:, kt, :],
                        start=(kt == 0), stop=(kt == KT - 1),
                    )
                gps = pp.tile([P, 1], f32, tag="pb")
                nc.tensor.matmul(gps[:], lhsT=xt[0][0:GATE_CH, tsl], rhs=wg_sb[:],
                                 start=True, stop=True)
                gcol = wk_pool.tile([P, 1], f32, tag="gcol")
                nc.scalar.activation(gcol[:], gps[:], AF.Sigmoid)
                # v = ve2 * sigmoid(g) + v_proj   (ve2 pre-scaled by 2 on host)
                nc.vector.scalar_tensor_tensor(
                    out=vsb[:, tt, :], in0=ve2_sb[:, tt, :], scalar=gcol[:],
                    in1=vps[:], op0=ALU.mult, op1=ALU.add,
                )

            # ---- attention (S^T tiles) + output projection -------------
            CO = C_ // 512  # output column chunks
            for qi in range(NT):
                ktc = min(WT + 1, NT - qi)
                qsl = slice(qi * P, (qi + 1) * P)
                denp = pp.tile([1, GH * P], f32, tag="pb")
                yu = []
                for h in range(GH):
                    # scores S^T = khat_tile.T @ qhat_tile, in chunks of <=4 kt
                    pts = []
                    for c0 in range(0, ktc, 4):
                        cw = min(4, ktc - c0)
                        sp = pp.tile([P, cw * P], f32, tag="pb")
                        for j in range(cw):
                            kt = qi + c0 + j
                            nc.tensor.matmul(
                                sp[:, j * P:(j + 1) * P],
                                lhsT=khat[:, kt * P:(kt + 1) * P],
                                rhs=qhat[:, h, qsl],
                                start=True, stop=True,
                            )
                        pt = aw.tile([P, cw * P], bf16, tag="pT")
                        nc.scalar.activation(pt[:], sp[:], AF.Exp)
                        pts.append((c0, cw, pt))
                    # band masks: diagonal tile keeps j>=i; far tile keeps j-i<win
                    pt0 = pts[0][2]
                    nc.vector.tensor_mul(pt0[:, 0:P], pt0[:, 0:P], tlo_sb[:])
                    if ktc == WT + 1:
                        cl, cwl, ptl = pts[-1]
                        jl = ktc - 1 - cl
                        nc.vector.tensor_mul(ptl[:, jl * P:(jl + 1) * P],
                                             ptl[:, jl * P:(jl + 1) * P], thi_sb[:])
                    # denominator row + PV accumulation
                    yp = pp.tile([P, HD], f32, tag="pb")
                    idx = 0
                    for (c0, cw, pt) in pts:
                        for j in range(cw):
                            kt = qi + c0 + j
                            nc.tensor.matmul(
                                denp[0:1, h * P:(h + 1) * P],
                                lhsT=ones_sb[:], rhs=pt[:, j * P:(j + 1) * P],
                                start=(idx == 0), stop=(idx == ktc - 1),
                            )
                            nc.tensor.matmul(
                                yp[:], lhsT=vsb[:, kt, :], rhs=pt[:, j * P:(j + 1) * P],
                                start=(idx == 0), stop=(idx == ktc - 1),
                            )
                            idx += 1
                    yut = aw.tile([P, HD], f32, tag="yu")
                    nc.vector.tensor_copy(yut[:], yp[:])
                    yu.append(yut)
                # normalize: y^T * (1/den) broadcast over d partitions
                rd = wk_pool.tile([1, GH * P], f32, tag="rd")
                nc.vector.reciprocal(rd[:], denp[:])
                db2 = drp.tile([1, GH * P], f32, tag="bounce")
                nc.sync.dma_start(out=db2[:], in_=rd[:])
                rdb = wk_pool.tile([P, GH * P], f32, tag="bcast")
                nc.sync.dma_start(out=rdb[:], in_=db2[:].to_broadcast((P, GH * P)))
                yq = aw.tile([P, GH, HD], bf16, tag="yq")
                for h in range(GH):
                    nc.vector.tensor_mul(yq[:, h, :], yu[h][:], rdb[:, h * P:(h + 1) * P])
                # out-proj: out[t, :] += sum_h yq_h.T @ Wo_h
                for co in range(CO):
                    osl = slice(co * 512, co * 512 + 512)
                    ops = pp.tile([P, 512], f32, tag="pb")
                    for h in range(GH):
                        nc.tensor.matmul(
                            ops[:], lhsT=yq[:, h, :], rhs=wo_sb[:, h, osl],
                            start=(h == 0), stop=(h == GH - 1),
                        )
                    ob = op_pool.tile([P, 512], f32, tag="ob")
                    nc.any.tensor_copy(out=ob[:], in_=ops[:])
                    nc.sync.dma_start(out=out_d[qsl, osl], in_=ob[:])

    return nc


def _get_program(T_=T, C_=C, win=WINDOW):
    key = (T_, C_, win)
    if key not in _PROGRAM_CACHE:
        nc = build_program(T_, C_, win)
        nc.finalize()
        _PROGRAM_CACHE[key] = nc
    return _PROGRAM_CACHE[key]


def make_in_maps(x, ve, cos, sin, Wq, Wk, Wv, Wg, Wo):
    """Build the 8 per-core input dicts (host-side sharding/layout prep)."""
    cosT = np.ascontiguousarray(cos[:, 0, :].T).astype(np.float32)  # [64, T]
    sinT = np.ascontiguousarray(sin[:, 0, :].T).astype(np.float32)
    cc = np.concatenate([cosT, cosT], axis=0)            # [128, T]
    ss = np.concatenate([sinT, -sinT], axis=0)           # [128, T]
    tlo = np.tril(np.ones((P, P), dtype=np.float32)).astype(BF16)      # keep j>=i
    thi = np.triu(np.ones((P, P), dtype=np.float32), 1).astype(BF16)   # keep j<i

    in_maps = []
    for core in range(N_CORES):
        b, g = divmod(core, N_KV)
        in_maps.append({
            "xT": np.ascontiguousarray(x[b].T).astype(BF16),
            "wq": Wq[:, g * GH * HD:(g + 1) * GH * HD].astype(BF16),
            "wk": Wk[:, g * HD:(g + 1) * HD].astype(BF16),
            "wv": Wv[:, g * HD:(g + 1) * HD].astype(BF16),
            "wg": np.ascontiguousarray(Wg[:, g:g + 1]).astype(BF16),
            "ve2": (2.0 * ve[b][:, g * HD:(g + 1) * HD]).astype(BF16),
            "wo": Wo[g * GH * HD:(g + 1) * GH * HD, :].astype(BF16),
            "cc": cc.astype(BF16), "ss": ss.astype(BF16),
            "tlo": tlo, "thi": thi,
        })
    return in_maps


def kernel(x, ve, cos, sin, Wq, Wk, Wv, Wg, Wo, window):
    assert int(window) == WINDOW and x.shape == (B, T, C)
    from concourse.bass_utils import run_bass_kernel_spmd

    nc = _get_program()
    in_maps = make_in_maps(x, ve, cos, sin, Wq, Wk, Wv, Wg, Wo)
    res = run_bass_kernel_spmd(nc, in_maps, core_ids=list(range(N_CORES)))
    out = np.zeros((B, T, C), dtype=np.float32)
    for core in range(N_CORES):
        b = core // N_KV
        out[b] += res.results[core]["out"]
    return out


# revision 21
# speedup vs baseline: 1.1153x; 1.1153x over previous
"""Trainium2 Bass kernel for a GQA sliding-window attention layer.

Reference computation (B=2, T=2048, C=2048, 16 Q heads / 4 KV heads, d=128):
    q = x @ Wq; k = x @ Wk; v = x @ Wv (+ sigmoid-gated value embedding)
    q, k = rmsnorm(rope(q)), rmsnorm(rope(k))
    scores masked to the band 0 <= j - i < window (=1024), softmax over j
    out = (p @ v) @ Wo

Sharding: 8 cores = 2 batches x 4 KV groups.  Each core computes its 4 Q
heads / 1 KV head for one batch and a partial output (its 512-row slice of
the Wo contraction); the host sums the 4 partials per batch.

Layout strategy per core:
  - xT (C x T, bf16) resident in SBUF; all projections contract over C.
  - q̂T / k̂T kept [d=128 partitions, T free]; scores computed transposed
    (S^T tiles [kj, qi]) so that P^T feeds the PV matmul directly with v in
    natural [token, d] layout (no P transposes).
  - softmax has no max-subtraction: rms-normalized q,k bound |score| by
    sqrt(128), so exp is safe in fp32.
  - per-q softmax denominators and rms rows are broadcast across partitions
    via a tiny DRAM bounce (SBUF APs need nonzero partition stride).
"""

import numpy as np
import ml_dtypes

BF16 = ml_dtypes.bfloat16

# Problem dims (hardcoded per contest rules)
B, T, C = 2, 2048, 2048
N_HEAD, N_KV, HD, GATE_CH = 16, 4, 128, 32
WINDOW = 1024
P = 128
GH = N_HEAD // N_KV  # q heads per kv head (= per core)
N_CORES = 8

_PROGRAM_CACHE = {}


def build_program(T_=T, C_=C, win=WINDOW):
    import concourse.mybir as mybir
    import concourse.tile as tile
    from concourse import bacc

    dt = mybir.dt
    f32 = dt.float32
    bf16 = dt.bfloat16
    AF = mybir.ActivationFunctionType
    ALU = mybir.AluOpType

    NT = T_ // P          # token tiles
    KT = C_ // P          # contraction tiles
    WT = win // P         # window tiles
    ISQ = 1.0 / float(np.sqrt(HD))

    nc = bacc.Bacc()

    xT = nc.declare_dram_parameter("xT", [C_, T_], bf16, isOutput=False)
    wq = nc.declare_dram_parameter("wq", [C_, GH * HD], bf16, isOutput=False)
    wk = nc.declare_dram_parameter("wk", [C_, HD], bf16, isOutput=False)
    wv = nc.declare_dram_parameter("wv", [C_, HD], bf16, isOutput=False)
    wg = nc.declare_dram_parameter("wg", [GATE_CH, 1], bf16, isOutput=False)
    ve2 = nc.declare_dram_parameter("ve2", [T_, HD], bf16, isOutput=False)
    wo = nc.declare_dram_parameter("wo", [GH * HD, C_], bf16, isOutput=False)
    ccd = nc.declare_dram_parameter("cc", [P, T_], bf16, isOutput=False)
    ssd = nc.declare_dram_parameter("ss", [P, T_], bf16, isOutput=False)
    tlo = nc.declare_dram_parameter("tlo", [P, P], bf16, isOutput=False)
    thi = nc.declare_dram_parameter("thi", [P, P], bf16, isOutput=False)
    idn = nc.declare_dram_parameter("ident", [P, P], bf16, isOutput=False)
    out_d = nc.declare_dram_parameter("out", [T_, C_], f32, isOutput=True)

    with tile.TileContext(nc) as tc:
        with (
            tc.tile_pool(name="singles", bufs=1) as sg,
            tc.tile_pool(name="work", bufs=2) as wk_pool,
            tc.tile_pool(name="attw", bufs=4) as aw,
            tc.tile_pool(name="outp", bufs=3) as op_pool,
            tc.tile_pool(name="psum", bufs=8, space="PSUM") as pp,
        ):
            # ---- persistent inputs -------------------------------------
            xt = []
            for kt in range(KT):
                t_ = sg.tile([P, T_], bf16, tag=f"xt{kt}")
                nc.sync.dma_start(out=t_[:], in_=xT[kt * P:(kt + 1) * P, :])
                xt.append(t_)
            wq_sb = sg.tile([P, KT, GH * HD], bf16, tag="wq")
            nc.sync.dma_start(out=wq_sb[:], in_=wq.rearrange("(o p) n -> p o n", p=P))
            wk_sb = sg.tile([P, KT, HD], bf16, tag="wk")
            nc.sync.dma_start(out=wk_sb[:], in_=wk.rearrange("(o p) n -> p o n", p=P))
            wv_sb = sg.tile([P, KT, HD], bf16, tag="wv")
            nc.sync.dma_start(out=wv_sb[:], in_=wv.rearrange("(o p) n -> p o n", p=P))
            wo_sb = sg.tile([P, GH, C_], bf16, tag="wo")
            nc.sync.dma_start(out=wo_sb[:], in_=wo.rearrange("(o p) n -> p o n", p=P))
            wg_sb = sg.tile([GATE_CH, 1], bf16, tag="wg")
            nc.sync.dma_start(out=wg_sb[:], in_=wg[:])
            cc_sb = sg.tile([P, T_], bf16, tag="cc")
            nc.sync.dma_start(out=cc_sb[:], in_=ccd[:])
            ss_sb = sg.tile([P, T_], bf16, tag="ss")
            nc.sync.dma_start(out=ss_sb[:], in_=ssd[:])
            ve2_sb = sg.tile([P, NT, HD], bf16, tag="ve2")
            nc.sync.dma_start(out=ve2_sb[:], in_=ve2.rearrange("(o p) d -> p o d", p=P))
            tlo_sb = sg.tile([P, P], bf16, tag="tlo")
            nc.sync.dma_start(out=tlo_sb[:], in_=tlo[:])
            thi_sb = sg.tile([P, P], bf16, tag="thi")
            nc.sync.dma_start(out=thi_sb[:], in_=thi[:])
            idn_sb = sg.tile([P, P], bf16, tag="idn")
            nc.sync.dma_start(out=idn_sb[:], in_=idn[:])
            ones_sb = sg.tile([P, 1], bf16, tag="onesb")
            nc.vector.memset(ones_sb[:], 1.0)
            ones1f = sg.tile([1, P], f32, tag="ones1f")
            nc.vector.memset(ones1f[:], 1.0)
            eps_sb = sg.tile([P, 1], f32, tag="epsb")
            nc.vector.memset(eps_sb[:], 1e-6)

            # persistent intermediates
            qhat = sg.tile([P, GH, T_], bf16, tag="qhat")   # normalized roped q, [d, h, t]
            khat = sg.tile([P, T_], bf16, tag="khat")       # normalized roped k * isq
            vsb = sg.tile([P, NT, HD], bf16, tag="vsb")     # gated v, [tok, tt, d]

            TS = T_ // 512  # 512-wide token slices

            # ---- projections + rope + rmsnorm for k and q heads --------
            # head index: 0 => K, 1..GH => Q h-1
            for head in range(GH + 1):
                is_k = head == 0
                for ts_ in range(TS):
                    sl = slice(ts_ * 512, ts_ * 512 + 512)
                    ps = pp.tile([P, 512], f32, tag="pb")
                    for kt in range(KT):
                        if is_k:
                            w_ap = wk_sb[:, kt, :]
                        else:
                            h = head - 1
                            w_ap = wq_sb[:, kt, h * HD:(h + 1) * HD]
                        nc.tensor.matmul(
                            ps[:], lhsT=w_ap, rhs=xt[kt][:, sl],
                            start=(kt == 0), stop=(kt == KT - 1),
                        )
                    # rope: qr = ps*cc + swap(ps)*ss   (ss carries the sign)
                    qr = wk_pool.tile([P, 512], f32, tag="qr")
                    nc.vector.tensor_mul(qr[:], ps[:], cc_sb[:, sl])
                    qs = wk_pool.tile([P, 512], f32, tag="qs")
                    nc.vector.tensor_mul(qs[0:64, :], ps[64:128, :], ss_sb[0:64, sl])
                    nc.vector.tensor_mul(qs[64:128, :], ps[0:64, :], ss_sb[64:128, sl])
                    nc.vector.tensor_add(qr[:], qr[:], qs[:])
                    # rms row: 1/sqrt(mean(qr^2)+eps)
                    q2 = wk_pool.tile([P, 512], bf16, tag="q2")
                    nc.vector.tensor_mul(q2[:], qr[:], qr[:])
                    ssq = pp.tile([1, 512], f32, tag="pb")
                    nc.tensor.matmul(ssq[:], lhsT=ones_sb[:], rhs=q2[:],
                                     start=True, stop=True)
                    srow = wk_pool.tile([1, 512], f32, tag="srow")
                    nc.scalar.activation(srow[:], ssq[:], AF.Sqrt,
                                         bias=eps_sb[0:1, :], scale=1.0 / HD)
                    rr = wk_pool.tile([1, 512], f32, tag="rr")
                    nc.vector.reciprocal_approx_fast(rr[:], srow[:])
                    if is_k:
                        # fold the 1/sqrt(d) score scale into k̂
                        nc.vector.tensor_scalar_mul(rr[:], rr[:], ISQ)
                    # broadcast rr across partitions: ones-column outer product
                    rrb = pp.tile([P, 512], f32, tag="pb")
                    nc.tensor.matmul(rrb[:], lhsT=ones1f[:], rhs=rr[:],
                                     start=True, stop=True)
                    dest = khat[:, sl] if is_k else qhat[:, head - 1, sl]
                    nc.vector.tensor_mul(dest, qr[:], rrb[:])

            # ---- v projection + sigmoid-gated value embedding ----------
            for tt in range(NT):
                tsl = slice(tt * P, (tt + 1) * P)
                vps = pp.tile([P, HD], f32, tag="pb")
                for kt in range(KT):
                    nc.tensor.matmul(
                        vps[:], lhsT=xt[kt][:, tsl], rhs=wv_sb[:, kt, :],
                        start=(kt == 0), stop=(kt == KT - 1),
                    )
                gps = pp.tile([P, 1], f32, tag="pb")
                nc.tensor.matmul(gps[:], lhsT=xt[0][0:GATE_CH, tsl], rhs=wg_sb[:],
                                 start=True, stop=True)
                gcol = wk_pool.tile([P, 1], f32, tag="gcol")
                nc.scalar.activation(gcol[:], gps[:], AF.Sigmoid)
                # v = ve2 * sigmoid(g) + v_proj   (ve2 pre-scaled by 2 on host)
                nc.vector.scalar_tensor_tensor(
                    out=vsb[:, tt, :], in0=ve2_sb[:, tt, :], scalar=gcol[:],
                    in1=vps[:], op0=ALU.mult, op1=ALU.add,
                )

            # ---- attention (S^T tiles) + output projection -------------
            CO = C_ // 512  # output column chunks
            for qi in range(NT):
                ktc = min(WT + 1, NT - qi)
                qsl = slice(qi * P, (qi + 1) * P)
                denp = pp.tile([1, GH * P], f32, tag="pb")
                yu = []
                for h in range(GH):
                    # scores S^T = khat_tile.T @ qhat_tile, in chunks of <=4 kt
                    pts = []
                    for c0 in range(0, ktc, 4):
                        cw = min(4, ktc - c0)
                        sp = pp.tile([P, cw * P], f32, tag="pb")
                        for j in range(cw):
                            kk = c0 + j
                            kt = qi + kk
                            # band-mask bias (-3e4 outside band) is folded in
                            # as a second accumulated matmul: bias.T @ I
                            masked = (kk == 0) or (kk == WT and ktc == WT + 1)
                            nc.tensor.matmul(
                                sp[:, j * P:(j + 1) * P],
                                lhsT=khat[:, kt * P:(kt + 1) * P],
                                rhs=qhat[:, h, qsl],
                                start=True, stop=not masked,
                            )
                            if masked:
                                nc.tensor.matmul(
                                    sp[:, j * P:(j + 1) * P],
                                    lhsT=tlo_sb[:] if kk == 0 else thi_sb[:],
                                    rhs=idn_sb[:],
                                    start=False, stop=True,
                                )
                        pt = aw.tile([P, cw * P], bf16, tag="pT")
                        nc.scalar.activation(pt[:], sp[:], AF.Exp)
                        pts.append((c0, cw, pt))
                    # denominator row + PV accumulation
                    yp = pp.tile([P, HD], f32, tag="pb")
                    idx = 0
                    for (c0, cw, pt) in pts:
                        for j in range(cw):
                            kt = qi + c0 + j
                            nc.tensor.matmul(
                                denp[0:1, h * P:(h + 1) * P],
                                lhsT=ones_sb[:], rhs=pt[:, j * P:(j + 1) * P],
                                start=(idx == 0), stop=(idx == ktc - 1),
                            )
                            nc.tensor.matmul(
                                yp[:], lhsT=vsb[:, kt, :], rhs=pt[:, j * P:(j + 1) * P],
                                start=(idx == 0), stop=(idx == ktc - 1),
                            )
                            idx += 1
                    yut = aw.tile([P, HD], f32, tag="yu")
                    nc.vector.tensor_copy(yut[:], yp[:])
                    yu.append(yut)
                # normalize: y^T * (1/den) broadcast over d partitions
                rd = wk_pool.tile([1, GH * P], f32, tag="rd")
                nc.vector.reciprocal_approx_fast(rd[:], denp[:])
                rdb = pp.tile([P, GH * P], f32, tag="pb")
                nc.tensor.matmul(rdb[:], lhsT=ones1f[:], rhs=rd[:],
                                 start=True, stop=True)
                yq = aw.tile([P, GH, HD], bf16, tag="yq")
                for h in range(GH):
                    nc.vector.tensor_mul(yq[:, h, :], yu[h][:], rdb[:, h * P:(h + 1) * P])
                # out-proj: out[t, :] += sum_h yq_h.T @ Wo_h
                for co in range(CO):
                    osl = slice(co * 512, co * 512 + 512)
                    ops = pp.tile([P, 512], f32, tag="pb")
                    for h in range(GH):
                        nc.tensor.matmul(
                            ops[:], lhsT=yq[:, h, :], rhs=wo_sb[:, h, osl],
                            start=(h == 0), stop=(h == GH - 1),
                        )
                    ob = op_pool.tile([P, 512], f32, tag="ob")
                    nc.any.tensor_copy(out=ob[:], in_=ops[:])
                    nc.sync.dma_start(out=out_d[qsl, osl], in_=ob[:])

    return nc


def _get_program(T_=T, C_=C, win=WINDOW):
    key = (T_, C_, win)
    if key not in _PROGRAM_CACHE:
        nc = build_program(T_, C_, win)
        nc.finalize()
        _PROGRAM_CACHE[key] = nc
    return _PROGRAM_CACHE[key]


def make_in_maps(x, ve, cos, sin, Wq, Wk, Wv, Wg, Wo):
    """Build the 8 per-core input dicts (host-side sharding/layout prep)."""
    cosT = np.ascontiguousarray(cos[:, 0, :].T).astype(np.float32)  # [64, T]
    sinT = np.ascontiguousarray(sin[:, 0, :].T).astype(np.float32)
    cc = np.concatenate([cosT, cosT], axis=0)            # [128, T]
    ss = np.concatenate([sinT, -sinT], axis=0)           # [128, T]
    # additive mask biases for the S^T diagonal/far tiles, pre-transposed
    # (they enter the scores as lhsT with an identity rhs: psum += bias.T)
    neg = np.float32(-30000.0)
    bias_lo = np.where(np.arange(P)[:, None] >= np.arange(P)[None, :], 0.0, neg)
    bias_hi = np.where(np.arange(P)[:, None] < np.arange(P)[None, :], 0.0, neg)
    tlo = np.ascontiguousarray(bias_lo.T).astype(BF16)
    thi = np.ascontiguousarray(bias_hi.T).astype(BF16)
    ident = np.eye(P, dtype=np.float32).astype(BF16)

    in_maps = []
    for core in range(N_CORES):
        b, g = divmod(core, N_KV)
        in_maps.append({
            "xT": np.ascontiguousarray(x[b].T).astype(BF16),
            "wq": Wq[:, g * GH * HD:(g + 1) * GH * HD].astype(BF16),
            "wk": Wk[:, g * HD:(g + 1) * HD].astype(BF16),
            "wv": Wv[:, g * HD:(g + 1) * HD].astype(BF16),
            "wg": np.ascontiguousarray(Wg[:, g:g + 1]).astype(BF16),
            "ve2": (2.0 * ve[b][:, g * HD:(g + 1) * HD]).astype(BF16),
            "wo": Wo[g * GH * HD:(g + 1) * GH * HD, :].astype(BF16),
            "cc": cc.astype(BF16), "ss": ss.astype(BF16),
            "tlo": tlo, "thi": thi, "ident": ident,
        })
    return in_maps


def kernel(x, ve, cos, sin, Wq, Wk, Wv, Wg, Wo, window):
    assert int(window) == WINDOW and x.shape == (B, T, C)
    from concourse.bass_utils import run_bass_kernel_spmd

    nc = _get_program()
    in_maps = make_in_maps(x, ve, cos, sin, Wq, Wk, Wv, Wg, Wo)
    res = run_bass_kernel_spmd(nc, in_maps, core_ids=list(range(N_CORES)))
    out = np.zeros((B, T, C), dtype=np.float32)
    for core in range(N_CORES):
        b = core // N_KV
        out[b] += res.results[core]["out"]
    return out


# revision 29
# speedup vs baseline: 1.5538x; 1.3931x over previous
"""Trainium2 Bass kernel for a GQA sliding-window attention layer.

Reference computation (B=2, T=2048, C=2048, 16 Q heads / 4 KV heads, d=128):
    q = x @ Wq; k = x @ Wk; v = x @ Wv (+ sigmoid-gated value embedding)
    q, k = rmsnorm(rope(q)), rmsnorm(rope(k))
    scores masked to the band 0 <= j - i < window (=1024), softmax over j
    out = (p @ v) @ Wo

Sharding: 8 cores = 2 batches x 4 KV groups.  Each core computes its 4 Q
heads / 1 KV head for one batch and a partial output (its 512-row slice of
the Wo contraction); the host sums the 4 partials per batch.

Layout strategy per core:
  - xT (C x T, bf16) resident in SBUF; all projections contract over C.
  - q̂T / k̂T kept [d=128 partitions, T free]; scores computed transposed
    (S^T tiles [kj, qi]) so that P^T feeds the PV matmul directly with v in
    natural [token, d] layout (no P transposes).
  - softmax has no max-subtraction: rms-normalized q,k bound |score| by
    sqrt(128), so exp is safe in fp32.
  - per-q softmax denominators and rms rows are broadcast across partitions
    via a tiny DRAM bounce (SBUF APs need nonzero partition stride).
"""

import numpy as np
import ml_dtypes

BF16 = ml_dtypes.bfloat16

# Problem dims (hardcoded per contest rules)
B, T, C = 2, 2048, 2048
N_HEAD, N_KV, HD, GATE_CH = 16, 4, 128, 32
WINDOW = 1024
P = 128
GH = N_HEAD // N_KV  # q heads per kv head (= per core)
N_CORES = 8

_PROGRAM_CACHE = {}


def build_program(T_=T, C_=C, win=WINDOW):
    import concourse.mybir as mybir
    import concourse.tile as tile
    from concourse import bacc

    dt = mybir.dt
    f32 = dt.float32
    bf16 = dt.bfloat16
    AF = mybir.ActivationFunctionType
    ALU = mybir.AluOpType

    NT = T_ // P          # token tiles
    KT = C_ // P          # contraction tiles
    WT = win // P         # window tiles
    ISQ = 1.0 / float(np.sqrt(HD))

    nc = bacc.Bacc()

    xT = nc.declare_dram_parameter("xT", [C_, T_], bf16, isOutput=False)
    wq = nc.declare_dram_parameter("wq", [C_, GH * HD], bf16, isOutput=False)
    wk = nc.declare_dram_parameter("wk", [C_, HD], bf16, isOutput=False)
    wv = nc.declare_dram_parameter("wv", [C_, HD], bf16, isOutput=False)
    wg = nc.declare_dram_parameter("wg", [GATE_CH, 1], bf16, isOutput=False)
    ve2 = nc.declare_dram_parameter("ve2", [T_, HD], bf16, isOutput=False)
    wo = nc.declare_dram_parameter("wo", [GH * HD, C_], bf16, isOutput=False)
    ccd = nc.declare_dram_parameter("cc", [P, T_], bf16, isOutput=False)
    ssd = nc.declare_dram_parameter("ss", [P, T_], bf16, isOutput=False)
    tlo = nc.declare_dram_parameter("tlo", [P, P], bf16, isOutput=False)
    thi = nc.declare_dram_parameter("thi", [P, P], bf16, isOutput=False)
    idn = nc.declare_dram_parameter("ident", [P, P], bf16, isOutput=False)
    out_d = nc.declare_dram_parameter("out", [T_, C_], f32, isOutput=True)

    with tile.TileContext(nc) as tc:
        with (
            tc.tile_pool(name="singles", bufs=1) as sg,
            tc.tile_pool(name="work", bufs=2) as wk_pool,
            tc.tile_pool(name="attw", bufs=8) as aw,
            tc.tile_pool(name="yup", bufs=8) as yu_pool,
            tc.tile_pool(name="outp", bufs=3) as op_pool,
            tc.tile_pool(name="psum", bufs=8, space="PSUM") as pp,
        ):
            # ---- persistent inputs -------------------------------------
            xt = []
            for kt in range(KT):
                t_ = sg.tile([P, T_], bf16, tag=f"xt{kt}")
                nc.sync.dma_start(out=t_[:], in_=xT[kt * P:(kt + 1) * P, :])
                xt.append(t_)
            wq_sb = sg.tile([P, KT, GH * HD], bf16, tag="wq")
            nc.sync.dma_start(out=wq_sb[:], in_=wq.rearrange("(o p) n -> p o n", p=P))
            wk_sb = sg.tile([P, KT, HD], bf16, tag="wk")
            nc.sync.dma_start(out=wk_sb[:], in_=wk.rearrange("(o p) n -> p o n", p=P))
            wv_sb = sg.tile([P, KT, HD], bf16, tag="wv")
            nc.sync.dma_start(out=wv_sb[:], in_=wv.rearrange("(o p) n -> p o n", p=P))
            wo_sb = sg.tile([P, GH, C_], bf16, tag="wo")
            nc.sync.dma_start(out=wo_sb[:], in_=wo.rearrange("(o p) n -> p o n", p=P))
            wg_sb = sg.tile([GATE_CH, 1], bf16, tag="wg")
            nc.sync.dma_start(out=wg_sb[:], in_=wg[:])
            cc_sb = sg.tile([P, T_], bf16, tag="cc")
            nc.sync.dma_start(out=cc_sb[:], in_=ccd[:])
            ss_sb = sg.tile([P, T_], bf16, tag="ss")
            nc.sync.dma_start(out=ss_sb[:], in_=ssd[:])
            ve2_sb = sg.tile([P, NT, HD], bf16, tag="ve2")
            nc.sync.dma_start(out=ve2_sb[:], in_=ve2.rearrange("(o p) d -> p o d", p=P))
            tlo_sb = sg.tile([P, P], bf16, tag="tlo")
            nc.sync.dma_start(out=tlo_sb[:], in_=tlo[:])
            thi_sb = sg.tile([P, P], bf16, tag="thi")
            nc.sync.dma_start(out=thi_sb[:], in_=thi[:])
            idn_sb = sg.tile([P, P], bf16, tag="idn")
            nc.sync.dma_start(out=idn_sb[:], in_=idn[:])
            ones_sb = sg.tile([P, 1], bf16, tag="onesb")
            nc.vector.memset(ones_sb[:], 1.0)
            ones1f = sg.tile([1, P], f32, tag="ones1f")
            nc.vector.memset(ones1f[:], 1.0)
            eps_sb = sg.tile([P, 1], f32, tag="epsb")
            nc.vector.memset(eps_sb[:], 1e-6)

            # persistent intermediates
            qhat = sg.tile([P, GH, T_], bf16, tag="qhat")   # normalized roped q, [d, h, t]
            khat = sg.tile([P, T_], bf16, tag="khat")       # normalized roped k * isq
            vsb = sg.tile([P, NT, HD], bf16, tag="vsb")     # gated v, [tok, tt, d]

            TS = T_ // 512  # 512-wide token slices

            # ---- projections + rope + rmsnorm for k and q heads --------
            # Software-pipelined by hand: the heavy 16-matmul accumulation of
            # iteration i+1 is emitted BEFORE iteration i's dependent tail
            # (rope / rms / broadcast), so the PE stream never stalls on the
            # DVE/ACT chain and HAM stays warm.
            def proj_head(head):
                is_k = head == 0
                out = []
                for ts_ in range(TS):
                    sl = slice(ts_ * 512, ts_ * 512 + 512)
                    ps = pp.tile([P, 512], f32, tag="pb")
                    for kt in range(KT):
                        if is_k:
                            w_ap = wk_sb[:, kt, :]
                        else:
                            h = head - 1
                            w_ap = wq_sb[:, kt, h * HD:(h + 1) * HD]
                        nc.tensor.matmul(
                            ps[:], lhsT=w_ap, rhs=xt[kt][:, sl],
                            start=(kt == 0), stop=(kt == KT - 1),
                        )
                    out.append((head, sl, ps))
                return out

            def proj_tail(head, sl, ps):
                is_k = head == 0
                # rope: qr = ps*cc + swap(ps)*ss   (ss carries the sign)
                qr = wk_pool.tile([P, 512], f32, tag="qr")
                nc.vector.tensor_mul(qr[:], ps[:], cc_sb[:, sl])
                qs = wk_pool.tile([P, 512], f32, tag="qs")
                nc.vector.tensor_mul(qs[0:64, :], ps[64:128, :], ss_sb[0:64, sl])
                nc.vector.tensor_mul(qs[64:128, :], ps[0:64, :], ss_sb[64:128, sl])
                nc.vector.tensor_add(qr[:], qr[:], qs[:])
                # rms row: 1/sqrt(mean(qr^2)+eps)
                q2 = wk_pool.tile([P, 512], bf16, tag="q2")
                nc.vector.tensor_mul(q2[:], qr[:], qr[:])
                ssq = pp.tile([1, 512], f32, tag="pb")
                nc.tensor.matmul(ssq[:], lhsT=ones_sb[:], rhs=q2[:],
                                 start=True, stop=True)
                srow = wk_pool.tile([1, 512], f32, tag="srow")
                nc.scalar.activation(srow[:], ssq[:], AF.Sqrt,
                                     bias=eps_sb[0:1, :], scale=1.0 / HD)
                rr = wk_pool.tile([1, 512], f32, tag="rr")
                nc.vector.reciprocal_approx_fast(rr[:], srow[:])
                if is_k:
                    # fold the 1/sqrt(d) score scale into k̂
                    nc.vector.tensor_scalar_mul(rr[:], rr[:], ISQ)
                # broadcast rr across partitions: ones-column outer product
                rrb = pp.tile([P, 512], f32, tag="pb")
                nc.tensor.matmul(rrb[:], lhsT=ones1f[:], rhs=rr[:],
                                 start=True, stop=True)
                dest = khat[:, sl] if is_k else qhat[:, head - 1, sl]
                nc.vector.tensor_mul(dest, qr[:], rrb[:])

            def v_head(tt):
                tsl = slice(tt * P, (tt + 1) * P)
                vps = pp.tile([P, HD], f32, tag="pb")
                for kt in range(KT):
                    nc.tensor.matmul(
                        vps[:], lhsT=xt[kt][:, tsl], rhs=wv_sb[:, kt, :],
                        start=(kt == 0), stop=(kt == KT - 1),
                    )
                gps = pp.tile([P, 1], f32, tag="pb")
                nc.tensor.matmul(gps[:], lhsT=xt[0][0:GATE_CH, tsl], rhs=wg_sb[:],
                                 start=True, stop=True)
                return (tt, vps, gps)

            def v_tail(tt, vps, gps):
                gcol = wk_pool.tile([P, 1], f32, tag="gcol")
                nc.scalar.activation(gcol[:], gps[:], AF.Sigmoid)
                # v = ve2 * sigmoid(g) + v_proj   (ve2 pre-scaled by 2 on host)
                nc.vector.scalar_tensor_tensor(
                    out=vsb[:, tt, :], in0=ve2_sb[:, tt, :], scalar=gcol[:],
                    in1=vps[:], op0=ALU.mult, op1=ALU.add,
                )

            from collections import deque
            pend = deque()
            for head in range(GH + 1):
                for item in proj_head(head):
                    pend.append(("p", item))
                    if len(pend) >= 3:
                        kind, it = pend.popleft()
                        proj_tail(*it) if kind == "p" else v_tail(*it)
            for tt in range(NT):
                pend.append(("v", v_head(tt)))
                if len(pend) >= 3:
                    kind, it = pend.popleft()
                    proj_tail(*it) if kind == "p" else v_tail(*it)
            while pend:
                kind, it = pend.popleft()
                proj_tail(*it) if kind == "p" else v_tail(*it)

            # ---- attention (S^T tiles) + output projection -------------
            # Also hand-pipelined: stage A = scores+exp, stage B = den+PV,
            # stage C (normalize + out-proj) of qi-1 is emitted after stage B
            # of qi so PE never waits on the softmax-denominator chain.
            CO = C_ // 512  # output column chunks

            def attn_scores_h(qi, h):
                ktc = min(WT + 1, NT - qi)
                qsl = slice(qi * P, (qi + 1) * P)
                pts = []
                for c0 in range(0, ktc, 4):
                    cw = min(4, ktc - c0)
                    sp = pp.tile([P, cw * P], f32, tag="pb")
                    for j in range(cw):
                        kk = c0 + j
                        kt = qi + kk
                        # band-mask bias (-3e4 outside band) folds in as a
                        # second accumulated matmul: bias.T @ I
                        masked = (kk == 0) or (kk == WT and ktc == WT + 1)
                        nc.tensor.matmul(
                            sp[:, j * P:(j + 1) * P],
                            lhsT=khat[:, kt * P:(kt + 1) * P],
                            rhs=qhat[:, h, qsl],
                            start=True, stop=not masked,
                        )
                        if masked:
                            nc.tensor.matmul(
                                sp[:, j * P:(j + 1) * P],
                                lhsT=tlo_sb[:] if kk == 0 else thi_sb[:],
                                rhs=idn_sb[:],
                                start=False, stop=True,
                            )
                    pt = aw.tile([P, cw * P], bf16, tag="pT")
                    nc.scalar.activation(pt[:], sp[:], AF.Exp)
                    pts.append((c0, cw, pt))
                return pts

            denps = {}
            yus = {}
            rds = {}

            def attn_pv_h(qi, h, pts):
                ktc = min(WT + 1, NT - qi)
                if h == 0:
                    denps[qi] = pp.tile([1, GH * P], f32, tag="pb", name=f"denp{qi}")
                    yus[qi] = []
                denp = denps[qi]
                yp = pp.tile([P, HD], f32, tag="pb")
                idx = 0
                for (c0, cw, pt) in pts:
                    for j in range(cw):
                        kt = qi + c0 + j
                        nc.tensor.matmul(
                            denp[0:1, h * P:(h + 1) * P],
                            lhsT=ones_sb[:], rhs=pt[:, j * P:(j + 1) * P],
                            start=(idx == 0), stop=(idx == ktc - 1),
                        )
                        nc.tensor.matmul(
                            yp[:], lhsT=vsb[:, kt, :],
                            rhs=pt[:, j * P:(j + 1) * P],
                            start=(idx == 0), stop=(idx == ktc - 1),
                        )
                        idx += 1
                yut = yu_pool.tile([P, HD], f32, tag="yu")
                nc.vector.tensor_copy(yut[:], yp[:])
                yus[qi].append(yut)
                if h == GH - 1:
                    rd = wk_pool.tile([1, GH * P], f32, tag="rd")
                    nc.vector.reciprocal_approx_fast(rd[:], denp[:])
                    rds[qi] = rd

            def attn_out(qi):
                qsl = slice(qi * P, (qi + 1) * P)
                rdb = pp.tile([P, GH * P], f32, tag="pb")
                nc.tensor.matmul(rdb[:], lhsT=ones1f[:], rhs=rds[qi][:],
                                 start=True, stop=True)
                yq = op_pool.tile([P, GH, HD], bf16, tag="yq")
                for h in range(GH):
                    nc.vector.tensor_mul(yq[:, h, :], yus[qi][h][:],
                                         rdb[:, h * P:(h + 1) * P])
                for co in range(CO):
                    osl = slice(co * 512, co * 512 + 512)
                    ops = pp.tile([P, 512], f32, tag="pb")
                    for h in range(GH):
                        nc.tensor.matmul(
                            ops[:], lhsT=yq[:, h, :], rhs=wo_sb[:, h, osl],
                            start=(h == 0), stop=(h == GH - 1),
                        )
                    ob = op_pool.tile([P, 512], f32, tag="ob")
                    nc.any.tensor_copy(out=ob[:], in_=ops[:])
                    nc.sync.dma_start(out=out_d[qsl, osl], in_=ob[:])

            pv_queue = deque()
            for qi in range(NT):
                for h in range(GH):
                    pts = attn_scores_h(qi, h)
                    if qi > 0 and h == 1:
                        attn_out(qi - 1)
                    if pv_queue:
                        attn_pv_h(*pv_queue.popleft())
                    pv_queue.append((qi, h, pts))
            while pv_queue:
                attn_pv_h(*pv_queue.popleft())
            attn_out(NT - 1)

    return nc


def _get_program(T_=T, C_=C, win=WINDOW):
    key = (T_, C_, win)
    if key not in _PROGRAM_CACHE:
        nc = build_program(T_, C_, win)
        nc.finalize()
        _PROGRAM_CACHE[key] = nc
    return _PROGRAM_CACHE[key]


def make_in_maps(x, ve, cos, sin, Wq, Wk, Wv, Wg, Wo):
    """Build the 8 per-core input dicts (host-side sharding/layout prep)."""
    cosT = np.ascontiguousarray(cos[:, 0, :].T).astype(np.float32)  # [64, T]
    sinT = np.ascontiguousarray(sin[:, 0, :].T).astype(np.float32)
    cc = np.concatenate([cosT, cosT], axis=0)            # [128, T]
    ss = np.concatenate([sinT, -sinT], axis=0)           # [128, T]
    # additive mask biases for the S^T diagonal/far tiles, pre-transposed
    # (they enter the scores as lhsT with an identity rhs: psum += bias.T)
    neg = np.float32(-30000.0)
    bias_lo = np.where(np.arange(P)[:, None] >= np.arange(P)[None, :], 0.0, neg)
    bias_hi = np.where(np.arange(P)[:, None] < np.arange(P)[None, :], 0.0, neg)
    tlo = np.ascontiguousarray(bias_lo.T).astype(BF16)
    thi = np.ascontiguousarray(bias_hi.T).astype(BF16)
    ident = np.eye(P, dtype=np.float32).astype(BF16)

    in_maps = []
    for core in range(N_CORES):
        b, g = divmod(core, N_KV)
        in_maps.append({
            "xT": np.ascontiguousarray(x[b].T).astype(BF16),
            "wq": Wq[:, g * GH * HD:(g + 1) * GH * HD].astype(BF16),
            "wk": Wk[:, g * HD:(g + 1) * HD].astype(BF16),
            "wv": Wv[:, g * HD:(g + 1) * HD].astype(BF16),
            "wg": np.ascontiguousarray(Wg[:, g:g + 1]).astype(BF16),
            "ve2": (2.0 * ve[b][:, g * HD:(g + 1) * HD]).astype(BF16),
            "wo": Wo[g * GH * HD:(g + 1) * GH * HD, :].astype(BF16),
            "cc": cc.astype(BF16), "ss": ss.astype(BF16),
            "tlo": tlo, "thi": thi, "ident": ident,
        })
    return in_maps


def kernel(x, ve, cos, sin, Wq, Wk, Wv, Wg, Wo, window):
    assert int(window) == WINDOW and x.shape == (B, T, C)
    from concourse.bass_utils import run_bass_kernel_spmd

    nc = _get_program()
    in_maps = make_in_maps(x, ve, cos, sin, Wq, Wk, Wv, Wg, Wo)
    res = run_bass_kernel_spmd(nc, in_maps, core_ids=list(range(N_CORES)))
    out = np.zeros((B, T, C), dtype=np.float32)
    for core in range(N_CORES):
        b = core // N_KV
        out[b] += res.results[core]["out"]
    return out


# revision 40
# speedup vs baseline: 1.7444x; 1.1227x over previous
"""Trainium2 Bass kernel for a GQA sliding-window attention layer.

Reference computation (B=2, T=2048, C=2048, 16 Q heads / 4 KV heads, d=128):
    q = x @ Wq; k = x @ Wk; v = x @ Wv (+ sigmoid-gated value embedding)
    q, k = rmsnorm(rope(q)), rmsnorm(rope(k))
    scores masked to the band 0 <= j - i < window (=1024), softmax over j
    out = (p @ v) @ Wo

Sharding: 8 cores = 2 batches x 4 KV groups.  Each core computes its 4 Q
heads / 1 KV head for one batch and a partial output (its 512-row slice of
the Wo contraction); the host sums the 4 partials per batch.

Layout strategy per core:
  - xT (C x T, bf16) resident in SBUF; all projections contract over C.
  - q̂T / k̂T kept [d=128 partitions, T free]; scores computed transposed
    (S^T tiles [kj, qi]) so that P^T feeds the PV matmul directly with v in
    natural [token, d] layout (no P transposes).
  - softmax has no max-subtraction: rms-normalized q,k bound |score| by
    sqrt(128), so exp is safe in fp32.
  - per-q softmax denominators and rms rows are broadcast across partitions
    via a tiny DRAM bounce (SBUF APs need nonzero partition stride).
"""

import numpy as np
import ml_dtypes

BF16 = ml_dtypes.bfloat16

# Problem dims (hardcoded per contest rules)
B, T, C = 2, 2048, 2048
N_HEAD, N_KV, HD, GATE_CH = 16, 4, 128, 32
WINDOW = 1024
P = 128
GH = N_HEAD // N_KV  # q heads per kv head (= per core)
N_CORES = 8

_PROGRAM_CACHE = {}


def build_program(T_=T, C_=C, win=WINDOW):
    import concourse.mybir as mybir
    import concourse.tile as tile
    from concourse import bacc

    dt = mybir.dt
    f32 = dt.float32
    bf16 = dt.bfloat16
    AF = mybir.ActivationFunctionType
    ALU = mybir.AluOpType

    NT = T_ // P          # token tiles
    KT = C_ // P          # contraction tiles
    WT = win // P         # window tiles
    ISQ = 1.0 / float(np.sqrt(HD))

    nc = bacc.Bacc()

    xT = nc.declare_dram_parameter("xT", [C_, T_], bf16, isOutput=False)
    wq = nc.declare_dram_parameter("wq", [C_, GH * HD], bf16, isOutput=False)
    wk = nc.declare_dram_parameter("wk", [C_, HD], bf16, isOutput=False)
    wv = nc.declare_dram_parameter("wv", [C_, HD], bf16, isOutput=False)
    wg = nc.declare_dram_parameter("wg", [GATE_CH, 1], bf16, isOutput=False)
    ve2 = nc.declare_dram_parameter("ve2", [T_, HD], bf16, isOutput=False)
    wo = nc.declare_dram_parameter("wo", [GH * HD, C_], bf16, isOutput=False)
    ccd = nc.declare_dram_parameter("cc", [P, T_], bf16, isOutput=False)
    ssd = nc.declare_dram_parameter("ss", [P, T_], bf16, isOutput=False)
    tlo = nc.declare_dram_parameter("tlo", [P, P], bf16, isOutput=False)
    thi = nc.declare_dram_parameter("thi", [P, P], bf16, isOutput=False)
    idr = nc.declare_dram_parameter("identr", [P, GH * P], bf16, isOutput=False)
    idf = nc.declare_dram_parameter("identf", [P, P], f32, isOutput=False)
    out_d = nc.declare_dram_parameter("out", [T_, C_], f32, isOutput=True)
    f32r = dt.float32r

    with tile.TileContext(nc) as tc:
        with (
            tc.tile_pool(name="singles", bufs=1) as sg,
            tc.tile_pool(name="work", bufs=2) as wk_pool,
            tc.tile_pool(name="attw", bufs=4) as aw,
            tc.tile_pool(name="yup", bufs=3) as yu_pool,
            tc.tile_pool(name="outp", bufs=3) as op_pool,
            tc.tile_pool(name="psum", bufs=8, space="PSUM") as pp,
        ):
            # ---- persistent inputs -------------------------------------
            # weight/x DMAs are split per k-tile and interleaved so the
            # first projection matmuls (kt=0) can start almost immediately
            xt = []
            wq_sb = sg.tile([P, KT, GH * HD], bf16, tag="wq")
            wk_sb = sg.tile([P, KT, HD], bf16, tag="wk")
            wv_sb = sg.tile([P, KT, HD], bf16, tag="wv")
            wqr = wq.rearrange("(o p) n -> p o n", p=P)
            wkr = wk.rearrange("(o p) n -> p o n", p=P)
            wvr = wv.rearrange("(o p) n -> p o n", p=P)
            for kt in range(KT):
                t_ = sg.tile([P, T_], bf16, tag=f"xt{kt}")
                nc.sync.dma_start(out=t_[:], in_=xT[kt * P:(kt + 1) * P, :])
                xt.append(t_)
                nc.sync.dma_start(out=wq_sb[:, kt, :], in_=wqr[:, kt, :])
                nc.sync.dma_start(out=wk_sb[:, kt, :], in_=wkr[:, kt, :])
                nc.sync.dma_start(out=wv_sb[:, kt, :], in_=wvr[:, kt, :])
            wo_sb = sg.tile([P, GH, C_], bf16, tag="wo")
            nc.sync.dma_start(out=wo_sb[:], in_=wo.rearrange("(o p) n -> p o n", p=P))
            wg_sb = sg.tile([GATE_CH, 1], bf16, tag="wg")
            nc.sync.dma_start(out=wg_sb[:], in_=wg[:])
            cc_sb = sg.tile([P, T_], bf16, tag="cc")
            nc.sync.dma_start(out=cc_sb[:], in_=ccd[:])
            ss_sb = sg.tile([P, T_], bf16, tag="ss")
            nc.sync.dma_start(out=ss_sb[:], in_=ssd[:])
            ve2_sb = sg.tile([P, NT, HD], bf16, tag="ve2")
            nc.sync.dma_start(out=ve2_sb[:], in_=ve2.rearrange("(o p) d -> p o d", p=P))
            tlo_sb = sg.tile([P, P], bf16, tag="tlo")
            nc.sync.dma_start(out=tlo_sb[:], in_=tlo[:])
            thi_sb = sg.tile([P, P], bf16, tag="thi")
            nc.sync.dma_start(out=thi_sb[:], in_=thi[:])
            idr_sb = sg.tile([P, GH * P], bf16, tag="idr")
            nc.sync.dma_start(out=idr_sb[:], in_=idr[:])
            idf_sb = sg.tile([P, P], f32, tag="idf")
            nc.sync.dma_start(out=idf_sb[:], in_=idf[:])
            ones_sb = sg.tile([P, 1], bf16, tag="onesb")
            nc.vector.memset(ones_sb[:], 1.0)
            ones1f = sg.tile([1, P], f32, tag="ones1f")
            nc.vector.memset(ones1f[:], 1.0)
            eps_sb = sg.tile([P, 1], f32, tag="epsb")
            nc.vector.memset(eps_sb[:], 1e-6)

            # persistent intermediates
            qhat = sg.tile([P, GH, T_], bf16, tag="qhat")   # normalized roped q, [d, h, t]
            khat = sg.tile([P, T_], bf16, tag="khat")       # normalized roped k * isq
            vsb = sg.tile([P, NT, HD], bf16, tag="vsb")     # gated v, [tok, tt, d]

            TS = T_ // 512  # 512-wide token slices

            # ---- projections + rope + rmsnorm for k and q heads --------
            # Software-pipelined by hand: the heavy 16-matmul accumulation of
            # iteration i+1 is emitted BEFORE iteration i's dependent tail
            # (rope / rms / broadcast), so the PE stream never stalls on the
            # DVE/ACT chain and HAM stays warm.
            def proj_head(head):
                # head: 0 = K, 1..GH = Q(h-1), GH+1 = V (computed as vT)
                out = []
                for ts_ in range(TS):
                    sl = slice(ts_ * 512, ts_ * 512 + 512)
                    ps = pp.tile([P, 512], f32, tag="pb")
                    for kt in range(KT):
                        if head == 0:
                            w_ap = wk_sb[:, kt, :]
                        elif head == GH + 1:
                            w_ap = wv_sb[:, kt, :]
                        else:
                            h = head - 1
                            w_ap = wq_sb[:, kt, h * HD:(h + 1) * HD]
                        nc.tensor.matmul(
                            ps[:], lhsT=w_ap, rhs=xt[kt][:, sl],
                            start=(kt == 0), stop=(kt == KT - 1),
                        )
                    out.append((head, sl, ps))
                return out

            def proj_tail(head, sl, ps):
                is_k = head == 0
                # rope: qr = ps*cc + swap(ps)*ss   (ss carries the sign)
                qr = wk_pool.tile([P, 512], f32, tag="qr")
                nc.vector.tensor_mul(qr[:], ps[:], cc_sb[:, sl])
                qs = wk_pool.tile([P, 512], f32, tag="qs")
                nc.vector.tensor_mul(qs[0:64, :], ps[64:128, :], ss_sb[0:64, sl])
                nc.vector.tensor_mul(qs[64:128, :], ps[0:64, :], ss_sb[64:128, sl])
                nc.vector.tensor_add(qr[:], qr[:], qs[:])
                # rms row: 1/sqrt(mean(qr^2)+eps)
                q2 = wk_pool.tile([P, 512], bf16, tag="q2")
                nc.vector.tensor_mul(q2[:], qr[:], qr[:])
                ssq = pp.tile([1, 512], f32, tag="pb")
                nc.tensor.matmul(ssq[:], lhsT=ones_sb[:], rhs=q2[:],
                                 start=True, stop=True)
                srow = wk_pool.tile([1, 512], f32, tag="srow")
                nc.scalar.activation(srow[:], ssq[:], AF.Sqrt,
                                     bias=eps_sb[0:1, :], scale=1.0 / HD)
                rr = wk_pool.tile([1, 512], f32, tag="rr")
                nc.vector.reciprocal_approx_fast(rr[:], srow[:])
                if is_k:
                    # fold the 1/sqrt(d) score scale into k̂
                    nc.vector.tensor_scalar_mul(rr[:], rr[:], ISQ)
                # broadcast rr across partitions: ones-column outer product
                # (f32r: full-rate on PE, ~1e-5 accurate — plenty for a scale)
                rrb = pp.tile([P, 512], f32, tag="pb")
                nc.tensor.matmul(rrb[:], lhsT=ones1f[:], rhs=rr[:],
                                 start=True, stop=True)
                dest = khat[:, sl] if is_k else qhat[:, head - 1, sl]
                nc.vector.tensor_mul(dest, qr[:], rrb[:])

            def v_tail(head, sl, ps):
                # vT psum [d, tok] -> sbuf f32, then PE-transpose each 128-tok
                # block to natural [tok, d] and add the sigmoid-gated ve.
                vt = wk_pool.tile([P, 512], f32, tag="vt")
                nc.vector.tensor_copy(vt[:], ps[:])
                for i in range(4):
                    tt = sl.start // P + i
                    tsl = slice(tt * P, (tt + 1) * P)
                    tp = pp.tile([P, P], f32, tag="pb")
                    nc.tensor.transpose(tp[:], vt[:, i * P:(i + 1) * P], idf_sb[:])
                    gps = pp.tile([P, 1], f32, tag="pb")
                    nc.tensor.matmul(gps[:], lhsT=xt[0][0:GATE_CH, tsl],
                                     rhs=wg_sb[:], start=True, stop=True)
                    gcol = wk_pool.tile([P, 1], f32, tag="gcol")
                    nc.scalar.activation(gcol[:], gps[:], AF.Sigmoid)
                    # v = ve2 * sigmoid(g) + v_proj (ve2 pre-scaled by 2)
                    nc.vector.scalar_tensor_tensor(
                        out=vsb[:, tt, :], in0=ve2_sb[:, tt, :], scalar=gcol[:],
                        in1=tp[:], op0=ALU.mult, op1=ALU.add,
                    )

            from collections import deque
            pend = deque()
            for head in range(GH + 2):
                for item in proj_head(head):
                    pend.append(item)
                    if len(pend) >= 3:
                        it = pend.popleft()
                        v_tail(*it) if it[0] == GH + 1 else proj_tail(*it)
            while pend:
                it = pend.popleft()
                v_tail(*it) if it[0] == GH + 1 else proj_tail(*it)

            # ---- attention (S^T tiles) + output projection -------------
            # Also hand-pipelined: stage A = scores+exp, stage B = den+PV,
            # stage C (normalize + out-proj) of qi-1 is emitted after stage B
            # of qi so PE never waits on the softmax-denominator chain.
            CO = C_ // 512  # output column chunks

            # All 4 q-heads are fused into one 512-wide moving operand:
            # scores / exp / den / PV are each ONE N=512 instruction per
            # (qi, kt), so LDWEIGHTS fully hides under the matmul stream.
            denps = {}
            yps = {}
            yus = {}
            rds = {}

            def attn_scores_k(qi, kk):
                ktc = min(WT + 1, NT - qi)
                qs4 = qhat[:, :, qi * P:(qi + 1) * P]   # [d, (h, q)] = 512 wide
                kt = qi + kk
                sp = pp.tile([P, GH * P], f32, tag="pb")
                masked = (kk == 0) or (kk == WT and ktc == WT + 1)
                nc.tensor.matmul(
                    sp[:], lhsT=khat[:, kt * P:(kt + 1) * P], rhs=qs4,
                    start=True, stop=not masked,
                )
                if masked:
                    # band-mask bias (-3e4 outside band): psum += bias.T @ I_rep
                    nc.tensor.matmul(
                        sp[:], lhsT=tlo_sb[:] if kk == 0 else thi_sb[:],
                        rhs=idr_sb[:], start=False, stop=True,
                    )
                pt = aw.tile([P, GH * P], bf16, tag="pT")
                nc.scalar.activation(pt[:], sp[:], AF.Exp)
                return pt

            def attn_pv_k(qi, kk, pt):
                ktc = min(WT + 1, NT - qi)
                if kk == 0:
                    denps[qi] = pp.tile([1, GH * P], f32, tag="pb",
                                        name=f"denp{qi}")
                    yps[qi] = pp.tile([P, GH * P], f32, tag="pb",
                                      name=f"yp{qi}")
                kt = qi + kk
                nc.tensor.matmul(
                    denps[qi][:], lhsT=ones_sb[:], rhs=pt[:],
                    start=(kk == 0), stop=(kk == ktc - 1),
                )
                nc.tensor.matmul(
                    yps[qi][:], lhsT=vsb[:, kt, :], rhs=pt[:],
                    start=(kk == 0), stop=(kk == ktc - 1),
                )
                if kk == ktc - 1:
                    yut = yu_pool.tile([P, GH * P], f32, tag="yu")
                    nc.vector.tensor_copy(yut[:], yps[qi][:])
                    yus[qi] = yut
                    rd = wk_pool.tile([1, GH * P], f32, tag="rd")
                    nc.vector.reciprocal_approx_fast(rd[:], denps[qi][:])
                    rds[qi] = rd

            def attn_out(qi):
                qsl = slice(qi * P, (qi + 1) * P)
                rdb = pp.tile([P, GH * P], f32, tag="pb")
                nc.tensor.matmul(rdb[:], lhsT=ones1f[:], rhs=rds[qi][:],
                                 start=True, stop=True)
                yq = op_pool.tile([P, GH * P], bf16, tag="yq")
                nc.vector.tensor_mul(yq[:], yus[qi][:], rdb[:])
                for co in range(CO):
                    osl = slice(co * 512, co * 512 + 512)
                    ops = pp.tile([P, 512], f32, tag="pb")
                    for h in range(GH):
                        nc.tensor.matmul(
                            ops[:], lhsT=yq[:, h * P:(h + 1) * P],
                            rhs=wo_sb[:, h, osl],
                            start=(h == 0), stop=(h == GH - 1),
                        )
                    ob = op_pool.tile([P, 512], f32, tag="ob")
                    nc.any.tensor_copy(out=ob[:], in_=ops[:])
                    nc.sync.dma_start(out=out_d[qsl, osl], in_=ob[:])

            pv_queue = deque()
            done_out = set()
            for qi in range(NT):
                ktc = min(WT + 1, NT - qi)
                for kk in range(ktc):
                    pt = attn_scores_k(qi, kk)
                    if pv_queue:
                        attn_pv_k(*pv_queue.popleft())
                    pv_queue.append((qi, kk, pt))
                    if qi > 0 and (qi - 1) in rds and (qi - 1) not in done_out:
                        done_out.add(qi - 1)
                        attn_out(qi - 1)
            while pv_queue:
                attn_pv_k(*pv_queue.popleft())
            attn_out(NT - 1)

    return nc


def _get_program(T_=T, C_=C, win=WINDOW):
    key = (T_, C_, win)
    if key not in _PROGRAM_CACHE:
        nc = build_program(T_, C_, win)
        nc.finalize()
        _PROGRAM_CACHE[key] = nc
    return _PROGRAM_CACHE[key]


def make_in_maps(x, ve, cos, sin, Wq, Wk, Wv, Wg, Wo):
    """Build the 8 per-core input dicts (host-side sharding/layout prep)."""
    cosT = np.ascontiguousarray(cos[:, 0, :].T).astype(np.float32)  # [64, T]
    sinT = np.ascontiguousarray(sin[:, 0, :].T).astype(np.float32)
    cc = np.concatenate([cosT, cosT], axis=0)            # [128, T]
    ss = np.concatenate([sinT, -sinT], axis=0)           # [128, T]
    # additive mask biases for the S^T diagonal/far tiles, pre-transposed
    # (they enter the scores as lhsT with an identity rhs: psum += bias.T)
    neg = np.float32(-30000.0)
    bias_lo = np.where(np.arange(P)[:, None] >= np.arange(P)[None, :], 0.0, neg)
    bias_hi = np.where(np.arange(P)[:, None] < np.arange(P)[None, :], 0.0, neg)
    tlo = np.ascontiguousarray(bias_lo.T).astype(BF16)
    thi = np.ascontiguousarray(bias_hi.T).astype(BF16)
    identr = np.tile(np.eye(P, dtype=np.float32), (1, GH)).astype(BF16)
    identf = np.eye(P, dtype=np.float32)

    in_maps = []
    for core in range(N_CORES):
        b, g = divmod(core, N_KV)
        in_maps.append({
            "xT": np.ascontiguousarray(x[b].T).astype(BF16),
            "wq": Wq[:, g * GH * HD:(g + 1) * GH * HD].astype(BF16),
            "wk": Wk[:, g * HD:(g + 1) * HD].astype(BF16),
            "wv": Wv[:, g * HD:(g + 1) * HD].astype(BF16),
            "wg": np.ascontiguousarray(Wg[:, g:g + 1]).astype(BF16),
            "ve2": (2.0 * ve[b][:, g * HD:(g + 1) * HD]).astype(BF16),
            "wo": Wo[g * GH * HD:(g + 1) * GH * HD, :].astype(BF16),
            "cc": cc.astype(BF16), "ss": ss.astype(BF16),
            "tlo": tlo, "thi": thi, "identr": identr, "identf": identf,
        })
    return in_maps


def kernel(x, ve, cos, sin, Wq, Wk, Wv, Wg, Wo, window):
    assert int(window) == WINDOW and x.shape == (B, T, C)
    from concourse.bass_utils import run_bass_kernel_spmd

    nc = _get_program()
    in_maps = make_in_maps(x, ve, cos, sin, Wq, Wk, Wv, Wg, Wo)
    res = run_bass_kernel_spmd(nc, in_maps, core_ids=list(range(N_CORES)))
    out = np.zeros((B, T, C), dtype=np.float32)
    for core in range(N_CORES):
        b = core // N_KV
        out[b] += res.results[core]["out"]
    return out


# revision 42
# speedup vs baseline: 1.7667x; 1.0128x over previous
"""Trainium2 Bass kernel for a GQA sliding-window attention layer.

Reference computation (B=2, T=2048, C=2048, 16 Q heads / 4 KV heads, d=128):
    q = x @ Wq; k = x @ Wk; v = x @ Wv (+ sigmoid-gated value embedding)
    q, k = rmsnorm(rope(q)), rmsnorm(rope(k))
    scores masked to the band 0 <= j - i < window (=1024), softmax over j
    out = (p @ v) @ Wo

Sharding: 8 cores = 2 batches x 4 KV groups.  Each core computes its 4 Q
heads / 1 KV head for one batch and a partial output (its 512-row slice of
the Wo contraction); the host sums the 4 partials per batch.

Layout strategy per core:
  - xT (C x T, bf16) resident in SBUF; all projections contract over C.
  - q̂T / k̂T kept [d=128 partitions, T free]; scores computed transposed
    (S^T tiles [kj, qi]) so that P^T feeds the PV matmul directly with v in
    natural [token, d] layout (no P transposes).
  - softmax has no max-subtraction: rms-normalized q,k bound |score| by
    sqrt(128), so exp is safe in fp32.
  - per-q softmax denominators and rms rows are broadcast across partitions
    via a tiny DRAM bounce (SBUF APs need nonzero partition stride).
"""

import numpy as np
import ml_dtypes

BF16 = ml_dtypes.bfloat16

# Problem dims (hardcoded per contest rules)
B, T, C = 2, 2048, 2048
N_HEAD, N_KV, HD, GATE_CH = 16, 4, 128, 32
WINDOW = 1024
P = 128
GH = N_HEAD // N_KV  # q heads per kv head (= per core)
N_CORES = 8

_PROGRAM_CACHE = {}


def build_program(T_=T, C_=C, win=WINDOW):
    import concourse.mybir as mybir
    import concourse.tile as tile
    from concourse import bacc

    dt = mybir.dt
    f32 = dt.float32
    bf16 = dt.bfloat16
    AF = mybir.ActivationFunctionType
    ALU = mybir.AluOpType

    NT = T_ // P          # token tiles
    KT = C_ // P          # contraction tiles
    WT = win // P         # window tiles
    ISQ = 1.0 / float(np.sqrt(HD))

    nc = bacc.Bacc()

    xT = nc.declare_dram_parameter("xT", [C_, T_], bf16, isOutput=False)
    wq = nc.declare_dram_parameter("wq", [C_, GH * HD], bf16, isOutput=False)
    wk = nc.declare_dram_parameter("wk", [C_, HD], bf16, isOutput=False)
    wv = nc.declare_dram_parameter("wv", [C_, HD], bf16, isOutput=False)
    wg = nc.declare_dram_parameter("wg", [GATE_CH, 1], bf16, isOutput=False)
    ve2 = nc.declare_dram_parameter("ve2", [T_, HD], bf16, isOutput=False)
    wo = nc.declare_dram_parameter("wo", [GH * HD, C_], bf16, isOutput=False)
    ccd = nc.declare_dram_parameter("cc", [P, T_], bf16, isOutput=False)
    ssd = nc.declare_dram_parameter("ss", [P, T_], bf16, isOutput=False)
    tlo = nc.declare_dram_parameter("tlo", [P, P], bf16, isOutput=False)
    thi = nc.declare_dram_parameter("thi", [P, P], bf16, isOutput=False)
    idr = nc.declare_dram_parameter("identr", [P, GH * P], bf16, isOutput=False)
    idf = nc.declare_dram_parameter("identf", [P, P], f32, isOutput=False)
    out_d = nc.declare_dram_parameter("out", [T_, C_], f32, isOutput=True)
    f32r = dt.float32r

    with tile.TileContext(nc) as tc:
        with (
            tc.tile_pool(name="singles", bufs=1) as sg,
            tc.tile_pool(name="work", bufs=2) as wk_pool,
            tc.tile_pool(name="attw", bufs=4) as aw,
            tc.tile_pool(name="yup", bufs=3) as yu_pool,
            tc.tile_pool(name="outp", bufs=3) as op_pool,
            tc.tile_pool(name="psum", bufs=8, space="PSUM") as pp,
        ):
            # ---- persistent inputs -------------------------------------
            # weight/x DMAs are split per k-tile and interleaved so the
            # first projection matmuls (kt=0) can start almost immediately
            xt = []
            wq_sb = sg.tile([P, KT, GH * HD], bf16, tag="wq")
            wk_sb = sg.tile([P, KT, HD], bf16, tag="wk")
            wv_sb = sg.tile([P, KT, HD], bf16, tag="wv")
            wqr = wq.rearrange("(o p) n -> p o n", p=P)
            wkr = wk.rearrange("(o p) n -> p o n", p=P)
            wvr = wv.rearrange("(o p) n -> p o n", p=P)
            for kt in range(KT):
                t_ = sg.tile([P, T_], bf16, tag=f"xt{kt}")
                nc.sync.dma_start(out=t_[:], in_=xT[kt * P:(kt + 1) * P, :])
                xt.append(t_)
                nc.sync.dma_start(out=wq_sb[:, kt, :], in_=wqr[:, kt, :])
                nc.sync.dma_start(out=wk_sb[:, kt, :], in_=wkr[:, kt, :])
                nc.sync.dma_start(out=wv_sb[:, kt, :], in_=wvr[:, kt, :])
            wo_sb = sg.tile([P, GH, C_], bf16, tag="wo")
            nc.sync.dma_start(out=wo_sb[:], in_=wo.rearrange("(o p) n -> p o n", p=P))
            wg_sb = sg.tile([GATE_CH, 1], bf16, tag="wg")
            nc.sync.dma_start(out=wg_sb[:], in_=wg[:])
            cc_sb = sg.tile([P, T_], bf16, tag="cc")
            nc.sync.dma_start(out=cc_sb[:], in_=ccd[:])
            ss_sb = sg.tile([P, T_], bf16, tag="ss")
            nc.sync.dma_start(out=ss_sb[:], in_=ssd[:])
            ve2_sb = sg.tile([P, NT, HD], bf16, tag="ve2")
            nc.sync.dma_start(out=ve2_sb[:], in_=ve2.rearrange("(o p) d -> p o d", p=P))
            tlo_sb = sg.tile([P, P], bf16, tag="tlo")
            nc.sync.dma_start(out=tlo_sb[:], in_=tlo[:])
            thi_sb = sg.tile([P, P], bf16, tag="thi")
            nc.sync.dma_start(out=thi_sb[:], in_=thi[:])
            idr_sb = sg.tile([P, GH * P], bf16, tag="idr")
            nc.sync.dma_start(out=idr_sb[:], in_=idr[:])
            idf_sb = sg.tile([P, P], f32, tag="idf")
            nc.sync.dma_start(out=idf_sb[:], in_=idf[:])
            ones_sb = sg.tile([P, 1], bf16, tag="onesb")
            nc.vector.memset(ones_sb[:], 1.0)
            ones1f = sg.tile([1, P], f32, tag="ones1f")
            nc.vector.memset(ones1f[:], 1.0)
            eps_sb = sg.tile([P, 1], f32, tag="epsb")
            nc.vector.memset(eps_sb[:], 1e-6)

            # persistent intermediates
            qhat = sg.tile([P, GH, T_], bf16, tag="qhat")   # normalized roped q, [d, h, t]
            khat = sg.tile([P, T_], bf16, tag="khat")       # normalized roped k * isq
            vsb = sg.tile([P, NT, HD], bf16, tag="vsb")     # gated v, [tok, tt, d]

            TS = T_ // 512  # 512-wide token slices

            # ---- projections + rope + rmsnorm for k and q heads --------
            # Software-pipelined by hand: the heavy 16-matmul accumulation of
            # iteration i+1 is emitted BEFORE iteration i's dependent tail
            # (rope / rms / broadcast), so the PE stream never stalls on the
            # DVE/ACT chain and HAM stays warm.
            def proj_head(head):
                # head: 0 = K, 1..GH = Q(h-1), GH+1 = V (computed as vT)
                out = []
                for ts_ in range(TS):
                    sl = slice(ts_ * 512, ts_ * 512 + 512)
                    ps = pp.tile([P, 512], f32, tag="pb")
                    for kt in range(KT):
                        if head == 0:
                            w_ap = wk_sb[:, kt, :]
                        elif head == GH + 1:
                            w_ap = wv_sb[:, kt, :]
                        else:
                            h = head - 1
                            w_ap = wq_sb[:, kt, h * HD:(h + 1) * HD]
                        nc.tensor.matmul(
                            ps[:], lhsT=w_ap, rhs=xt[kt][:, sl],
                            start=(kt == 0), stop=(kt == KT - 1),
                        )
                    out.append((head, sl, ps))
                return out

            def proj_tail(head, sl, ps):
                is_k = head == 0
                # rope: qr = ps*cc + swap(ps)*ss   (ss carries the sign)
                qr = wk_pool.tile([P, 512], f32, tag="qr")
                nc.vector.tensor_mul(qr[:], ps[:], cc_sb[:, sl])
                qs = wk_pool.tile([P, 512], f32, tag="qs")
                nc.vector.tensor_mul(qs[0:64, :], ps[64:128, :], ss_sb[0:64, sl])
                nc.vector.tensor_mul(qs[64:128, :], ps[0:64, :], ss_sb[64:128, sl])
                nc.vector.tensor_add(qr[:], qr[:], qs[:])
                # rms row: 1/sqrt(mean(qr^2)+eps)
                q2 = wk_pool.tile([P, 512], bf16, tag="q2")
                nc.vector.tensor_mul(q2[:], qr[:], qr[:])
                ssq = pp.tile([1, 512], f32, tag="pb")
                nc.tensor.matmul(ssq[:], lhsT=ones_sb[:], rhs=q2[:],
                                 start=True, stop=True)
                srow = wk_pool.tile([1, 512], f32, tag="srow")
                nc.scalar.activation(srow[:], ssq[:], AF.Sqrt,
                                     bias=eps_sb[0:1, :], scale=1.0 / HD)
                rr = wk_pool.tile([1, 512], f32, tag="rr")
                nc.vector.reciprocal_approx_fast(rr[:], srow[:])
                if is_k:
                    # fold the 1/sqrt(d) score scale into k̂
                    nc.vector.tensor_scalar_mul(rr[:], rr[:], ISQ)
                # broadcast rr across partitions: ones-column outer product
                # (f32r: full-rate on PE, ~1e-5 accurate — plenty for a scale)
                rrb = pp.tile([P, 512], f32, tag="pb")
                nc.tensor.matmul(rrb[:], lhsT=ones1f[:], rhs=rr[:],
                                 start=True, stop=True)
                dest = khat[:, sl] if is_k else qhat[:, head - 1, sl]
                nc.vector.tensor_mul(dest, qr[:], rrb[:])

            def v_tail(head, sl, ps):
                # vT psum [d, tok] -> sbuf f32, then PE-transpose each 128-tok
                # block to natural [tok, d] and add the sigmoid-gated ve.
                vt = wk_pool.tile([P, 512], f32, tag="vt")
                nc.vector.tensor_copy(vt[:], ps[:])
                for i in range(4):
                    tt = sl.start // P + i
                    tsl = slice(tt * P, (tt + 1) * P)
                    tp = pp.tile([P, P], f32, tag="pb")
                    nc.tensor.transpose(tp[:], vt[:, i * P:(i + 1) * P], idf_sb[:])
                    gps = pp.tile([P, 1], f32, tag="pb")
                    nc.tensor.matmul(gps[:], lhsT=xt[0][0:GATE_CH, tsl],
                                     rhs=wg_sb[:], start=True, stop=True)
                    gcol = wk_pool.tile([P, 1], f32, tag="gcol")
                    nc.scalar.activation(gcol[:], gps[:], AF.Sigmoid)
                    # v = ve2 * sigmoid(g) + v_proj (ve2 pre-scaled by 2)
                    nc.vector.scalar_tensor_tensor(
                        out=vsb[:, tt, :], in0=ve2_sb[:, tt, :], scalar=gcol[:],
                        in1=tp[:], op0=ALU.mult, op1=ALU.add,
                    )

            from collections import deque
            pend = deque()
            for head in range(GH + 2):
                for item in proj_head(head):
                    pend.append(item)
                    if len(pend) >= 3:
                        it = pend.popleft()
                        v_tail(*it) if it[0] == GH + 1 else proj_tail(*it)
            while pend:
                it = pend.popleft()
                v_tail(*it) if it[0] == GH + 1 else proj_tail(*it)

            # ---- attention (S^T tiles) + output projection -------------
            # Also hand-pipelined: stage A = scores+exp, stage B = den+PV,
            # stage C (normalize + out-proj) of qi-1 is emitted after stage B
            # of qi so PE never waits on the softmax-denominator chain.
            CO = C_ // 512  # output column chunks

            # All 4 q-heads are fused into one 512-wide moving operand:
            # scores / exp / den / PV are each ONE N=512 instruction per
            # (qi, kt), so LDWEIGHTS fully hides under the matmul stream.
            denps = {}
            yps = {}
            yus = {}
            rds = {}

            def attn_scores_k(qi, kk):
                ktc = min(WT + 1, NT - qi)
                qs4 = qhat[:, :, qi * P:(qi + 1) * P]   # [d, (h, q)] = 512 wide
                kt = qi + kk
                sp = pp.tile([P, GH * P], f32, tag="pb")
                masked = (kk == 0) or (kk == WT and ktc == WT + 1)
                nc.tensor.matmul(
                    sp[:], lhsT=khat[:, kt * P:(kt + 1) * P], rhs=qs4,
                    start=True, stop=not masked,
                )
                if masked:
                    # band-mask bias (-3e4 outside band): psum += bias.T @ I_rep
                    nc.tensor.matmul(
                        sp[:], lhsT=tlo_sb[:] if kk == 0 else thi_sb[:],
                        rhs=idr_sb[:], start=False, stop=True,
                    )
                pt = aw.tile([P, GH * P], bf16, tag="pT")
                nc.scalar.activation(pt[:], sp[:], AF.Exp)
                return pt

            def attn_pv_k(qi, kk, pt):
                ktc = min(WT + 1, NT - qi)
                if kk == 0:
                    denps[qi] = pp.tile([1, GH * P], f32, tag="pb",
                                        name=f"denp{qi}")
                    yps[qi] = pp.tile([P, GH * P], f32, tag="pb",
                                      name=f"yp{qi}")
                kt = qi + kk
                nc.tensor.matmul(
                    denps[qi][:], lhsT=ones_sb[:], rhs=pt[:],
                    start=(kk == 0), stop=(kk == ktc - 1),
                )
                nc.tensor.matmul(
                    yps[qi][:], lhsT=vsb[:, kt, :], rhs=pt[:],
                    start=(kk == 0), stop=(kk == ktc - 1),
                )
                if kk == ktc - 1:
                    yut = yu_pool.tile([P, GH * P], f32, tag="yu")
                    nc.vector.tensor_copy(yut[:], yps[qi][:])
                    yus[qi] = yut
                    rd = wk_pool.tile([1, GH * P], f32, tag="rd")
                    nc.vector.reciprocal_approx_fast(rd[:], denps[qi][:])
                    rds[qi] = rd

            def attn_out(qi):
                qsl = slice(qi * P, (qi + 1) * P)
                rdb = pp.tile([P, GH * P], f32, tag="pb")
                nc.tensor.matmul(rdb[:], lhsT=ones1f[:], rhs=rds[qi][:],
                                 start=True, stop=True)
                yq = op_pool.tile([P, GH * P], bf16, tag="yq")
                nc.vector.tensor_mul(yq[:], yus[qi][:], rdb[:])
                for co in range(CO):
                    osl = slice(co * 512, co * 512 + 512)
                    ops = pp.tile([P, 512], f32, tag="pb")
                    for h in range(GH):
                        nc.tensor.matmul(
                            ops[:], lhsT=yq[:, h * P:(h + 1) * P],
                            rhs=wo_sb[:, h, osl],
                            start=(h == 0), stop=(h == GH - 1),
                        )
                    ob = op_pool.tile([P, 512], f32, tag="ob")
                    nc.any.tensor_copy(out=ob[:], in_=ops[:])
                    nc.sync.dma_start(out=out_d[qsl, osl], in_=ob[:])

            pv_queue = deque()
            done_out = set()
            for qi in range(NT):
                ktc = min(WT + 1, NT - qi)
                for kk in range(ktc):
                    pt = attn_scores_k(qi, kk)
                    if len(pv_queue) >= 2:
                        attn_pv_k(*pv_queue.popleft())
                    pv_queue.append((qi, kk, pt))
                    if qi > 0 and (qi - 1) in rds and (qi - 1) not in done_out:
                        done_out.add(qi - 1)
                        attn_out(qi - 1)
            while pv_queue:
                attn_pv_k(*pv_queue.popleft())
            for qi in range(NT):
                if qi not in done_out:
                    attn_out(qi)

    return nc


def _get_program(T_=T, C_=C, win=WINDOW):
    key = (T_, C_, win)
    if key not in _PROGRAM_CACHE:
        nc = build_program(T_, C_, win)
        nc.finalize()
        _PROGRAM_CACHE[key] = nc
    return _PROGRAM_CACHE[key]


def make_in_maps(x, ve, cos, sin, Wq, Wk, Wv, Wg, Wo):
    """Build the 8 per-core input dicts (host-side sharding/layout prep)."""
    cosT = np.ascontiguousarray(cos[:, 0, :].T).astype(np.float32)  # [64, T]
    sinT = np.ascontiguousarray(sin[:, 0, :].T).astype(np.float32)
    cc = np.concatenate([cosT, cosT], axis=0)            # [128, T]
    ss = np.concatenate([sinT, -sinT], axis=0)           # [128, T]
    # additive mask biases for the S^T diagonal/far tiles, pre-transposed
    # (they enter the scores as lhsT with an identity rhs: psum += bias.T)
    neg = np.float32(-30000.0)
    bias_lo = np.where(np.arange(P)[:, None] >= np.arange(P)[None, :], 0.0, neg)
    bias_hi = np.where(np.arange(P)[:, None] < np.arange(P)[None, :], 0.0, neg)
    tlo = np.ascontiguousarray(bias_lo.T).astype(BF16)
    thi = np.ascontiguousarray(bias_hi.T).astype(BF16)
    identr = np.tile(np.eye(P, dtype=np.float32), (1, GH)).astype(BF16)
    identf = np.eye(P, dtype=np.float32)

    in_maps = []
    for core in range(N_CORES):
        b, g = divmod(core, N_KV)
        in_maps.append({
            "xT": np.ascontiguousarray(x[b].T).astype(BF16),
            "wq": Wq[:, g * GH * HD:(g + 1) * GH * HD].astype(BF16),
            "wk": Wk[:, g * HD:(g + 1) * HD].astype(BF16),
            "wv": Wv[:, g * HD:(g + 1) * HD].astype(BF16),
            "wg": np.ascontiguousarray(Wg[:, g:g + 1]).astype(BF16),
            "ve2": (2.0 * ve[b][:, g * HD:(g + 1) * HD]).astype(BF16),
            "wo": Wo[g * GH * HD:(g + 1) * GH * HD, :].astype(BF16),
            "cc": cc.astype(BF16), "ss": ss.astype(BF16),
            "tlo": tlo, "thi": thi, "identr": identr, "identf": identf,
        })
    return in_maps


def kernel(x, ve, cos, sin, Wq, Wk, Wv, Wg, Wo, window):
    assert int(window) == WINDOW and x.shape == (B, T, C)
    from concourse.bass_utils import run_bass_kernel_spmd

    nc = _get_program()
    in_maps = make_in_maps(x, ve, cos, sin, Wq, Wk, Wv, Wg, Wo)
    res = run_bass_kernel_spmd(nc, in_maps, core_ids=list(range(N_CORES)))
    out = np.zeros((B, T, C), dtype=np.float32)
    for core in range(N_CORES):
        b = core // N_KV
        out[b] += res.results[core]["out"]
    return out
